# revision 1
# baseline (speedup 1.0000x reference)
"""SAGAN-style attention block (nn_AttentionBlock) on 8 Trainium2 NeuronCores.

Math (per batch b):
    q = wq @ x + bq            [C8, N]
    k = wk @ x + bk            [C8, N]
    v = wv @ x + bv            [C,  N]
    S[n, m]  = sum_o q[o,n] k[o,m]
    attn     = softmax_m(S)
    out[c,n] = sum_m v[c,m] attn[n,m]
    y        = gamma * out + x

Sharding: 8 cores = 4 batches x 2 halves of the n (query-row) axis.  Each
core holds the full x[b] (for K/V) plus its own n-slice (for Q + residual).

Per-core layout (channels/rows on partitions):
    S^T tiles [m(128 part), n(512)] via matmul(lhsT=k_tile, rhs=q_block)
    P^T = exp(S^T)  (no max subtraction: |S| <~ 40, safe in fp32/bf16)
    out[c,n]   = sum over 32 m-tiles of matmul(lhsT=vT[m,c], rhs=P^T[m,n])
    denom[1,n] = matmul(lhsT=ones[128,1], rhs=P^T)  accumulated the same way
    y = out * (gamma/denom broadcast) + x

QK^T/projection matmuls run as float32r (full-rate fp32 PE mode); the
AV/denominator path runs in bf16 (P^T is written by Exp directly as bf16).
"""

import sys

sys.path.insert(0, "/opt/trn_rl_repo")

import numpy as np  # noqa: E402

B, C, HH, WW = 4, 256, 64, 64
N = HH * WW  # 4096
C8 = C // 8  # 32
P = 128
CT = C // P  # 2 channel tiles
NQ = N // 2  # 2048 query rows per core
NBLK = 512  # n-block (query columns per block)
NBLKS = NQ // NBLK  # 4
MT = N // P  # 32 m-tiles (key/value positions)
CHUNK = 512
NCHUNKS = N // CHUNK  # 8
QCHUNKS = NQ // CHUNK  # 4
NCORES = 8

_prog = None


def _build(bench_reps=None, variant="full", exp_from_sbuf=False):
    import contextlib

    import concourse.bacc as bacc
    import concourse.mybir as mybir
    import concourse.tile as tile

    f32 = mybir.dt.float32
    AluAdd = mybir.AluOpType.add
    f32r = mybir.dt.float32r
    bf16 = mybir.dt.bfloat16
    Exp = mybir.ActivationFunctionType.Exp

    nc = bacc.Bacc("TRN2", target_bir_lowering=False, debug=False)

    xh_d = nc.dram_tensor("xh", [C, N], f32r, kind="ExternalInput")
    xq_d = nc.dram_tensor("xq", [C, NQ], f32r, kind="ExternalInput")
    # wqt/wkt are zero-padded on the host from [C, C8] to [C, 128] so the
    # projection matmul writes all 128 partitions of q/k (zero rows included)
    wqt_d = nc.dram_tensor("wqt", [C, P], f32r, kind="ExternalInput")
    wkt_d = nc.dram_tensor("wkt", [C, P], f32r, kind="ExternalInput")
    wvt_d = nc.dram_tensor("wvt", [C, C], f32r, kind="ExternalInput")
    bq_d = nc.dram_tensor("bq", [P], f32, kind="ExternalInput")
    bk_d = nc.dram_tensor("bk", [P], f32, kind="ExternalInput")
    bv_d = nc.dram_tensor("bv", [C], f32, kind="ExternalInput")
    g_d = nc.dram_tensor("gamma", [1], f32, kind="ExternalInput")
    ones32_d = nc.dram_tensor("ones32", [P], f32r, kind="ExternalInput")
    out_d = nc.dram_tensor("out", [C, NQ], f32, kind="ExternalOutput")

    with tile.TileContext(nc) as tc:
        with (
            tc.tile_pool(name="const", bufs=1) as const,
            tc.tile_pool(name="big", bufs=1) as big,
        ):
            # persistent SBUF tensors
            xh = big.tile([P, CT, N], f32r)  # x[b], channels on partitions
            xq = big.tile([P, CT, NQ], f32r)  # this core's n-slice of x[b]
            k_sb = big.tile([P, N], f32r)  # k, zero rows 32..127
            q_sb = big.tile([P, NQ], f32r)  # q rows 0..31; replicated to all
            # 4 partition groups for row-packed QK^T
            k_pk = big.tile([P, MT // 4, P], f32r)  # k m-tile 4s+g at [32g:, s, :]
            vt_sb = big.tile([P, MT, C], bf16)  # v^T tiles [m, c]

            wqt = const.tile([P, CT, P], f32r)
            wkt = const.tile([P, CT, P], f32r)
            wvt = const.tile([P, CT, C], f32r)
            bq_sb = const.tile([P, 1], f32)
            bk_sb = const.tile([P, 1], f32)
            bvb = const.tile([P, C], f32)  # bv broadcast over partitions
            gam = const.tile([1, 1], f32)
            ones_bf = const.tile([P, 1], bf16)
            ones1 = const.tile([1, P], f32)
            ones32 = const.tile([P, 1], f32r)

            nc.sync.dma_start(out=wqt, in_=wqt_d.ap().rearrange("(t p) o -> p t o", p=P))
            nc.sync.dma_start(out=wkt, in_=wkt_d.ap().rearrange("(t p) o -> p t o", p=P))
            nc.sync.dma_start(out=wvt, in_=wvt_d.ap().rearrange("(t p) o -> p t o", p=P))
            nc.sync.dma_start(out=bq_sb, in_=bq_d.ap()[:, None])
            nc.sync.dma_start(out=bk_sb, in_=bk_d.ap()[:, None])
            nc.gpsimd.dma_start(out=bvb, in_=bv_d.ap()[None, :].to_broadcast([P, C]))
            nc.sync.dma_start(out=gam, in_=g_d.ap()[:, None])
            nc.sync.dma_start(out=ones32, in_=ones32_d.ap()[:, None])
            nc.vector.memset(ones_bf, 1.0)
            nc.vector.memset(ones1, 1.0)

            xh_r = xh_d.ap().rearrange("(t p) n -> p t n", p=P)
            xq_r = xq_d.ap().rearrange("(t p) n -> p t n", p=P)
            out_r = out_d.ap().rearrange("(t p) n -> p t n", p=P)

            loop_ctx = (
                tc.For_i(0, bench_reps, 1)
                if bench_reps is not None
                else contextlib.nullcontext()
            )
            loop_ctx.__enter__()

            for ch in range(NCHUNKS):
                sl = slice(ch * CHUNK, (ch + 1) * CHUNK)
                nc.sync.dma_start(out=xh[:, :, sl], in_=xh_r[:, :, sl])
            for ch in range(QCHUNKS):
                sl = slice(ch * CHUNK, (ch + 1) * CHUNK)
                nc.sync.dma_start(out=xq[:, :, sl], in_=xq_r[:, :, sl])

            # ---- phase A: q/k/v projections ----
            with tc.tile_pool(name="pa_psum", bufs=2, space="PSUM") as pap:
                for ch in range(NCHUNKS):
                    sl = slice(ch * CHUNK, (ch + 1) * CHUNK)
                    kp = pap.tile([P, CHUNK], f32, tag="kq", name="kp")
                    for t in range(CT):
                        nc.tensor.matmul(
                            kp,
                            lhsT=wkt[:, t, :],
                            rhs=xh[:, t, sl],
                            start=(t == 0),
                            stop=(t == CT - 1),
                        )
                    nc.vector.tensor_scalar_add(k_sb[:, sl], kp, bk_sb)
                    for g in range(4):
                        mt = 4 * ch + g
                        nc.sync.dma_start(
                            out=k_pk[32 * g : 32 * g + 32, ch, :],
                            in_=k_sb[:C8, mt * P : (mt + 1) * P],
                        )
                for ch in range(QCHUNKS):
                    sl = slice(ch * CHUNK, (ch + 1) * CHUNK)
                    qp = pap.tile([P, CHUNK], f32, tag="kq", name="qp")
                    for t in range(CT):
                        nc.tensor.matmul(
                            qp,
                            lhsT=wqt[:, t, :],
                            rhs=xq[:, t, sl],
                            start=(t == 0),
                            stop=(t == CT - 1),
                        )
                    nc.vector.tensor_scalar_add(q_sb[:, sl], qp, bq_sb)
                    for g in range(1, 4):
                        nc.sync.dma_start(
                            out=q_sb[32 * g : 32 * g + 32, sl], in_=q_sb[:C8, sl]
                        )
                for mt in range(MT):
                    msl = slice(mt * P, (mt + 1) * P)
                    vp = pap.tile([P, C], f32, tag="v", name="vp")
                    for t in range(CT):
                        nc.tensor.matmul(
                            vp,
                            lhsT=xh[:, t, msl],
                            rhs=wvt[:, t, :],
                            start=(t == 0),
                            stop=(t == CT - 1),
                        )
                    # drain + bias + cast to bf16 in one DVE op
                    nc.vector.tensor_add(out=vt_sb[:, mt, :], in0=vp, in1=bvb)

            # ---- phase B: attention ----
            GRP = 4  # m-tiles per S^T psum group (4 banks, one per row group)
            with (
                tc.tile_pool(name="st_psum", bufs=1, space="PSUM") as stp,
                tc.tile_pool(name="acc_psum", bufs=1, space="PSUM") as accp,
                tc.tile_pool(name="pt_pool", bufs=2) as ptp,
                tc.tile_pool(name="fin_pool", bufs=3) as finp,
                tc.tile_pool(name="stsb_pool", bufs=2) as stsbp,
            ):
                if True:
                    NSLOT = MT // GRP  # S^T/exp slots per block
                    bstate = {}  # nb -> (nsl, pt, out_ps, den_ps)

                    def emit_av(nb, mg):
                        """AV matmuls + split denominator (PE half / DVE half)."""
                        if variant not in ("full", "av"):
                            return
                        nsl, pt, out_ps, den_ps, dacc = bstate[nb]
                        for i in range(GRP):
                            mt = GRP * mg + i
                            for cc in range(CT):
                                nc.tensor.matmul(
                                    out_ps[cc],
                                    lhsT=vt_sb[:, mt, cc * P : (cc + 1) * P],
                                    rhs=pt[:, mt, :],
                                    start=(mt == 0),
                                    stop=(mt == MT - 1),
                                )
                            if i == 0:
                                nc.tensor.matmul(
                                    den_ps,
                                    lhsT=ones_bf,
                                    rhs=pt[:, mt, :],
                                    start=(mt == 0),
                                    stop=False,
                                )
                            elif mt == GRP * 0 + 1:  # first DVE m-tile: init
                                nc.vector.tensor_copy(out=dacc, in_=pt[:, mt, :])
                            else:
                                nc.vector.tensor_tensor(
                                    dacc, dacc, pt[:, mt, :], AluAdd
                                )

                    def emit_tail(nb):
                        """Drain PSUM eagerly, then normalize + residual + store.

                        y = out * (gamma/denom) + x.  The PSUM->SBUF copies come
                        first so the accumulator banks free up for the next
                        block's matmuls; the rest overlaps next-block PE work.
                        """
                        if variant != "full":
                            return
                        nsl, pt, out_ps, den_ps, dacc = bstate.pop(nb)
                        nc.tensor.matmul(
                            den_ps,
                            lhsT=ones32,
                            rhs=dacc,
                            start=False,
                            stop=True,
                        )
                        rec = finp.tile([1, NBLK], f32, tag="rec", name="rec")
                        nc.vector.reciprocal(rec, den_ps)
                        nc.vector.tensor_scalar_mul(rec, rec, gam)
                        outc = []
                        for cc in range(CT):
                            oc = finp.tile([P, NBLK], f32, tag=f"oc{cc}", name="oc")
                            nc.vector.tensor_copy(out=oc, in_=out_ps[cc])
                            outc.append(oc)
                        bc_sb = finp.tile([P, NBLK], f32, tag="bcs", name="bc_sb")
                        nc.gpsimd.partition_broadcast(bc_sb, rec)
                        for cc in range(CT):
                            fin = finp.tile([P, NBLK], f32, tag="fin", name="fin")
                            nc.vector.tensor_mul(out=fin, in0=outc[cc], in1=bc_sb)
                            nc.vector.tensor_add(
                                out=fin, in0=fin, in1=xq[:, cc, nsl].bitcast(f32)
                            )
                            nc.sync.dma_start(out=out_r[:, cc, nsl], in_=fin)

                    # software-pipelined emission: the AV/den matmuls for slot
                    # s-1 are emitted between S^T(s) and its exp, so the PE
                    # never sits idle waiting for the ACT engine's exp
                    prev = None
                    for nb in range(NBLKS):
                        nsl = slice(nb * NBLK, (nb + 1) * NBLK)
                        pt = ptp.tile([P, MT, NBLK], bf16, tag="pt", name="pt")
                        out_ps0 = accp.tile([P, NBLK], f32, tag="out0", name="out_ps0")
                        out_ps1 = accp.tile([P, NBLK], f32, tag="out1", name="out_ps1")
                        den_ps = accp.tile([1, NBLK], f32, tag="den", name="den_ps")
                        dacc = finp.tile([P, NBLK], f32r, tag="dacc", name="dacc")
                        bstate[nb] = (nsl, pt, [out_ps0, out_ps1], den_ps, dacc)
                        for mg in range(NSLOT):
                            if variant in ("full", "qk"):
                                st = stp.tile([P, GRP, NBLK], f32, tag="st", name="st")
                                for g in range(GRP):
                                    nc.tensor.matmul(
                                        st[:, g, :],
                                        lhsT=k_pk[32 * g : 32 * g + 32, mg, :],
                                        rhs=q_sb[32 * g : 32 * g + 32, nsl],
                                        start=True,
                                        stop=True,
                                        tile_position=(32 * g, 0),
                                    )
                                if exp_from_sbuf:
                                    st_sb = stsbp.tile(
                                        [P, GRP, NBLK], f32, tag="stsb", name="st_sb"
                                    )
                                    nc.vector.tensor_copy(out=st_sb, in_=st)
                                    nc.scalar.activation(
                                        out=pt[:, GRP * mg : GRP * (mg + 1), :],
                                        in_=st_sb,
                                        func=Exp,
                                    )
                                else:
                                    nc.scalar.activation(
                                        out=pt[:, GRP * mg : GRP * (mg + 1), :],
                                        in_=st,
                                        func=Exp,
                                    )
                            if prev is not None:
                                pnb, pmg = prev
                                emit_av(pnb, pmg)
                                if pmg == NSLOT - 1:
                                    emit_tail(pnb)
                            prev = (nb, mg)
                    if prev is not None:
                        pnb, pmg = prev
                        emit_av(pnb, pmg)
                        emit_tail(pnb)

            loop_ctx.__exit__(None, None, None)

    nc.compile()
    return nc


def _get_prog():
    global _prog
    if _prog is None:
        _prog = _build()
    return _prog


def make_in_maps(inputs):
    x = np.ascontiguousarray(inputs["x"], dtype=np.float32).reshape(B, C, N)
    wqt = np.zeros((C, P), np.float32)
    wqt[:, :C8] = np.asarray(inputs["wq"], np.float32).T
    wkt = np.zeros((C, P), np.float32)
    wkt[:, :C8] = np.asarray(inputs["wk"], np.float32).T
    wvt = np.ascontiguousarray(np.asarray(inputs["wv"], np.float32).T)
    bq = np.zeros(P, np.float32)
    bq[:C8] = np.asarray(inputs["bq"], np.float32)
    bk = np.zeros(P, np.float32)
    bk[:C8] = np.asarray(inputs["bk"], np.float32)
    bv = np.ascontiguousarray(np.asarray(inputs["bv"], np.float32))
    gamma = np.ascontiguousarray(np.asarray(inputs["gamma"], np.float32).reshape(1))
    in_maps = []
    for core in range(NCORES):
        b, h = divmod(core, 2)
        in_maps.append(
            {
                "xh": x[b],
                "xq": np.ascontiguousarray(x[b][:, h * NQ : (h + 1) * NQ]),
                "wqt": wqt,
                "wkt": wkt,
                "wvt": wvt,
                "bq": bq,
                "bk": bk,
                "bv": bv,
                "gamma": gamma,
                "ones32": np.ones(P, np.float32),
            }
        )
    return in_maps


def assemble(results):
    out = np.empty((B, C, N), np.float32)
    for core in range(NCORES):
        b, h = divmod(core, 2)
        out[b][:, h * NQ : (h + 1) * NQ] = results[core]["out"]
    return out.reshape(B, C, HH, WW)


def kernel(**inputs):
    from concourse.bass_utils import run_bass_kernel_spmd

    nc = _get_prog()
    in_maps = make_in_maps(inputs)
    res = run_bass_kernel_spmd(nc, in_maps, core_ids=list(range(NCORES)))
    return assemble(res.results)



# revision 12
# speedup vs baseline: 1.3420x; 1.3420x over previous
"""SAGAN-style attention block (nn_AttentionBlock) on 8 Trainium2 NeuronCores.

Math (per batch b):
    q = wq @ x + bq            [C8, N]
    k = wk @ x + bk            [C8, N]
    v = wv @ x + bv            [C,  N]
    S[n, m]  = sum_o q[o,n] k[o,m]
    attn     = softmax_m(S)
    out[c,n] = sum_m v[c,m] attn[n,m]
    y        = gamma * out + x

Sharding: 8 cores = 4 batches x 2 halves of the n (query-row) axis.

v2 design notes (all chosen from trace evidence on v1):
  - whole PE path in bf16: host casts x to bf16 (fp32 x kept only for the
    residual add), weights bf16.  fp32(HIGH) matmuls self-load weights and
    cost ~2.4x.
  - wq/wk are host-tiled 4x across PE row-groups (wkt4[c, 32g+o] = wk[o,c])
    so the projection itself replicates k/q into all four 32-row partition
    groups: the quadrant-packed QK^T matmuls (tile_position=(32g,0), which
    run concurrently on the PE) then slice k_rep/q_rep directly - no
    SBUF->SBUF packing DMAs at all.
  - gamma is folded into wv on the host; gamma*bv is folded into the xq
    residual input on the host.  v-projection bias work disappears.
  - q/k biases ride the projection matmul as an extra rank-1 accumulation
    (lhsT = bias row [1,128], rhs = ones row) - zero DVE cost.
  - softmax denominator: per-slot bf16 running sum on DVE (one [128,2048]
    2x-mode add per slot), folded 4->1 on DVE, partition-summed AND
    broadcast in one gpsimd.partition_all_reduce, inverted with the fast
    custom-DVE reciprocal (the [1,512] iterative reciprocal in v1 cost 4us
    per block and stalled the PE at every block boundary).
  - PSUM: 4 banks S^T (single buffer) + 2x2 banks for the out accumulators
    (block-alternating), so block nb+1's matmuls start while block nb's
    tail drains.
"""

import sys

sys.path.insert(0, "/opt/trn_rl_repo")

import numpy as np  # noqa: E402

B, C, HH, WW = 4, 256, 64, 64
N = HH * WW  # 4096
C8 = C // 8  # 32
P = 128
CT = C // P  # 2 channel tiles
NQ = N // 2  # 2048 query rows per core
NBLK = 512  # n-block (query columns per block)
NBLKS = NQ // NBLK  # 4
MT = N // P  # 32 m-tiles (key/value positions)
GRP = 4  # m-tiles per S^T psum slot
NSLOT = MT // GRP  # 8 slots per block
CHUNK = 512
NCHUNKS = N // CHUNK  # 8
QCHUNKS = NQ // CHUNK  # 4
NCORES = 8

_prog = None


def _build(debug_taps=False):
    import concourse.bacc as bacc
    import concourse.bass_isa as bass_isa
    import concourse.mybir as mybir
    import concourse.tile as tile

    f32 = mybir.dt.float32
    f16 = mybir.dt.float16
    bf16 = mybir.dt.bfloat16
    AluAdd = mybir.AluOpType.add
    Exp = mybir.ActivationFunctionType.Exp
    RAdd = bass_isa.ReduceOp.add

    nc = bacc.Bacc("TRN2", target_bir_lowering=False, debug=False)

    dbg = {}
    if debug_taps:
        dbg["k"] = nc.dram_tensor("dbg_k", [P, N], f16, kind="ExternalOutput")
        dbg["q"] = nc.dram_tensor("dbg_q", [P, NQ], f16, kind="ExternalOutput")
        dbg["vt"] = nc.dram_tensor("dbg_vt", [P, MT * C], bf16, kind="ExternalOutput")
        dbg["pt"] = nc.dram_tensor("dbg_pt", [P, GRP * NBLK], bf16, kind="ExternalOutput")
        dbg["dacc"] = nc.dram_tensor("dbg_dacc", [P, GRP * NBLK], bf16, kind="ExternalOutput")
        dbg["dbc"] = nc.dram_tensor("dbg_dbc", [P, NBLK], f32, kind="ExternalOutput")
        dbg["rec"] = nc.dram_tensor("dbg_rec", [P, NBLK], f32, kind="ExternalOutput")
        dbg["acc0"] = nc.dram_tensor("dbg_acc0", [P, NBLK], f32, kind="ExternalOutput")

    xh_d = nc.dram_tensor("xh", [C, N], f16, kind="ExternalInput")
    xqh_d = nc.dram_tensor("xqh", [C, NQ], f16, kind="ExternalInput")
    xq_d = nc.dram_tensor("xq", [C, NQ], f32, kind="ExternalInput")
    wqt4_d = nc.dram_tensor("wqt4", [C, P], f16, kind="ExternalInput")
    wkt4_d = nc.dram_tensor("wkt4", [C, P], f16, kind="ExternalInput")
    wvt_d = nc.dram_tensor("wvt", [C, C], f16, kind="ExternalInput")
    bq4_d = nc.dram_tensor("bq4", [P], f32, kind="ExternalInput")
    bk4_d = nc.dram_tensor("bk4", [P], f32, kind="ExternalInput")
    out_d = nc.dram_tensor("out", [C, NQ], f32, kind="ExternalOutput")

    with tile.TileContext(nc) as tc:
        with (
            tc.tile_pool(name="const", bufs=1) as const,
            tc.tile_pool(name="big", bufs=1) as big,
        ):
            xh = big.tile([P, CT, N], f16)
            xqh = big.tile([P, CT, NQ], f16)
            xq = big.tile([P, CT, NQ], f32)
            k_rep = big.tile([P, N], f16)  # k replicated in 4 row groups
            q_rep = big.tile([P, NQ], f16)
            vt = big.tile([P, MT, C], bf16)  # v^T tiles [m, c], gamma-scaled

            wqt4 = const.tile([P, CT, P], f16)
            wkt4 = const.tile([P, CT, P], f16)
            wvt = const.tile([P, CT, C], f16)
            bq4 = const.tile([P, 1], f32)
            bk4 = const.tile([P, 1], f32)

            nc.sync.dma_start(out=wqt4, in_=wqt4_d.ap().rearrange("(t p) o -> p t o", p=P))
            nc.sync.dma_start(out=wkt4, in_=wkt4_d.ap().rearrange("(t p) o -> p t o", p=P))
            nc.sync.dma_start(out=wvt, in_=wvt_d.ap().rearrange("(t p) o -> p t o", p=P))
            nc.sync.dma_start(out=bq4, in_=bq4_d.ap()[:, None])
            nc.sync.dma_start(out=bk4, in_=bk4_d.ap()[:, None])

            xh_r = xh_d.ap().rearrange("(t p) n -> p t n", p=P)
            xqh_r = xqh_d.ap().rearrange("(t p) n -> p t n", p=P)
            xq_r = xq_d.ap().rearrange("(t p) n -> p t n", p=P)
            out_r = out_d.ap().rearrange("(t p) n -> p t n", p=P)

            for ch in range(NCHUNKS):
                sl = slice(ch * CHUNK, (ch + 1) * CHUNK)
                nc.sync.dma_start(out=xh[:, :, sl], in_=xh_r[:, :, sl])
            for ch in range(QCHUNKS):
                sl = slice(ch * CHUNK, (ch + 1) * CHUNK)
                nc.sync.dma_start(out=xqh[:, :, sl], in_=xqh_r[:, :, sl])
            for ch in range(QCHUNKS):
                sl = slice(ch * CHUNK, (ch + 1) * CHUNK)
                nc.gpsimd.dma_start(out=xq[:, :, sl], in_=xq_r[:, :, sl])

            # ---- phase A: q/k/v projections (all bf16 on the PE) ----
            with tc.tile_pool(name="pa", bufs=2, space="PSUM") as pap:
                def proj_chunk(dst, w4, bcol, src_x, sl):
                    pp = pap.tile([P, CHUNK], f32, tag="pj", name="pp")
                    for t in range(CT):
                        nc.tensor.matmul(
                            pp, lhsT=w4[:, t, :], rhs=src_x[:, t, sl],
                            start=(t == 0), stop=(t == CT - 1),
                        )
                    # fused drain + per-partition bias add + f16 cast
                    nc.vector.tensor_scalar_add(dst, pp, bcol)

                for ch in range(NCHUNKS):
                    sl = slice(ch * CHUNK, (ch + 1) * CHUNK)
                    proj_chunk(k_rep[:, sl], wkt4, bk4, xh, sl)
                for ch in range(QCHUNKS):
                    sl = slice(ch * CHUNK, (ch + 1) * CHUNK)
                    proj_chunk(q_rep[:, sl], wqt4, bq4, xqh, sl)
                for mt in range(MT):
                    msl = slice(mt * P, (mt + 1) * P)
                    vp = pap.tile([P, CHUNK], f32, tag="pj", name="vp")
                    for t in range(CT):
                        nc.tensor.matmul(
                            vp[:, :C], lhsT=xh[:, t, msl], rhs=wvt[:, t, :],
                            start=(t == 0), stop=(t == CT - 1),
                        )
                    nc.vector.tensor_copy(out=vt[:, mt, :], in_=vp[:, :C])
                if debug_taps:
                    nc.sync.dma_start(out=dbg["k"].ap(), in_=k_rep)
                    nc.sync.dma_start(out=dbg["q"].ap(), in_=q_rep)
                    nc.sync.dma_start(out=dbg["vt"].ap().rearrange("p (m c) -> p m c", m=MT), in_=vt)

            # ---- phase B: attention ----
            with (
                tc.tile_pool(name="st_ps", bufs=1, space="PSUM") as stp,
                tc.tile_pool(name="acc_ps", bufs=2, space="PSUM") as accp,
                tc.tile_pool(name="ptp", bufs=3) as ptp,
                tc.tile_pool(name="dap", bufs=2) as dap,
                tc.tile_pool(name="dnp", bufs=2) as dnp,
                tc.tile_pool(name="finp", bufs=4) as finp,
            ):
                bstate = {}

                def emit_av(nb, mg, pt):
                    accs, dacc, nsl = bstate[nb]
                    for i in range(GRP):
                        mt = GRP * mg + i
                        for cc in range(CT):
                            nc.tensor.matmul(
                                accs[cc],
                                lhsT=vt[:, mt, cc * P:(cc + 1) * P],
                                rhs=pt[:, i, :],
                                start=(mt == 0),
                                stop=(mt == MT - 1),
                            )
                    # denominator partial: one 2048-elem bf16 add per slot
                    if mg == 0:
                        nc.vector.tensor_copy(out=dacc, in_=pt)
                    else:
                        nc.vector.tensor_tensor(dacc, dacc, pt, AluAdd)

                def emit_tail(nb):
                    accs, dacc, nsl = bstate.pop(nb)
                    d2 = dnp.tile([P, 2, NBLK], bf16, tag="d2", name="d2")
                    nc.vector.tensor_tensor(d2, dacc[:, 0:2, :], dacc[:, 2:4, :], AluAdd)
                    d1 = dnp.tile([P, NBLK], bf16, tag="d1", name="d1")
                    nc.vector.tensor_tensor(d1, d2[:, 0, :], d2[:, 1, :], AluAdd)
                    # sum over partitions, result broadcast to all partitions
                    dbc = dnp.tile([P, NBLK], f32, tag="dbc", name="dbc")
                    nc.gpsimd.partition_all_reduce(dbc, d1, channels=P, reduce_op=RAdd)
                    rec = dnp.tile([P, NBLK], f32, tag="rec", name="rec")
                    nc.vector.reciprocal_approx_fast(rec, dbc)
                    if debug_taps and nb == 0:
                        nc.sync.dma_start(out=dbg["dacc"].ap().rearrange("p (g n) -> p g n", g=GRP), in_=dacc)
                        nc.sync.dma_start(out=dbg["dbc"].ap(), in_=dbc)
                        nc.sync.dma_start(out=dbg["rec"].ap(), in_=rec)
                        acc_sb = finp.tile([P, NBLK], f32, tag="fin", name="accsb")
                        nc.vector.tensor_copy(out=acc_sb, in_=accs[0])
                        nc.sync.dma_start(out=dbg["acc0"].ap(), in_=acc_sb)
                    for cc in range(CT):
                        fin = finp.tile([P, NBLK], f32, tag="fin", name="fin")
                        nc.vector.tensor_mul(out=fin, in0=accs[cc], in1=rec)
                        nc.vector.tensor_add(out=fin, in0=fin, in1=xq[:, cc, nsl])
                        nc.sync.dma_start(out=out_r[:, cc, nsl], in_=fin)

                prev = None
                for nb in range(NBLKS):
                    nsl = slice(nb * NBLK, (nb + 1) * NBLK)
                    a0 = accp.tile([P, NBLK], f32, tag="o0", name="a0")
                    a1 = accp.tile([P, NBLK], f32, tag="o1", name="a1")
                    dacc = dap.tile([P, GRP, NBLK], bf16, tag="da", name="dacc")
                    bstate[nb] = ([a0, a1], dacc, nsl)
                    for mg in range(NSLOT):
                        st = stp.tile([P, GRP, NBLK], f32, tag="st", name="st")
                        for g in range(GRP):
                            mt = GRP * mg + g
                            nc.tensor.matmul(
                                st[:, g, :],
                                lhsT=k_rep[32 * g:32 * g + 32, mt * P:(mt + 1) * P],
                                rhs=q_rep[32 * g:32 * g + 32, nsl],
                                start=True,
                                stop=True,
                                tile_position=(32 * g, 0),
                            )
                        pt = ptp.tile([P, GRP, NBLK], bf16, tag="pt", name="pt")
                        nc.scalar.activation(out=pt, in_=st, func=Exp)
                        if debug_taps and nb == 0 and mg == 0:
                            nc.sync.dma_start(out=dbg["pt"].ap().rearrange("p (g n) -> p g n", g=GRP), in_=pt)
                        if prev is not None:
                            pnb, pmg, ppt = prev
                            emit_av(pnb, pmg, ppt)
                            if pmg == NSLOT - 1:
                                emit_tail(pnb)
                        prev = (nb, mg, pt)
                pnb, pmg, ppt = prev
                emit_av(pnb, pmg, ppt)
                emit_tail(pnb)

    nc.compile()
    return nc


def _get_prog():
    global _prog
    if _prog is None:
        _prog = _build()
    return _prog


def make_in_maps(inputs):
    import ml_dtypes

    bf = ml_dtypes.bfloat16
    x = np.ascontiguousarray(inputs["x"], dtype=np.float32).reshape(B, C, N)
    gamma = float(np.asarray(inputs["gamma"], np.float32).reshape(()))
    wq = np.asarray(inputs["wq"], np.float32)
    wk = np.asarray(inputs["wk"], np.float32)
    wv = np.asarray(inputs["wv"], np.float32)
    bq = np.asarray(inputs["bq"], np.float32)
    bk = np.asarray(inputs["bk"], np.float32)
    bv = np.asarray(inputs["bv"], np.float32)

    wqt4 = np.ascontiguousarray(np.tile(wq.T, (1, 4)).astype(np.float16))  # [C,128]
    wkt4 = np.ascontiguousarray(np.tile(wk.T, (1, 4)).astype(np.float16))
    wvt = np.ascontiguousarray((gamma * wv.T).astype(np.float16))  # [C,C]
    bq4 = np.ascontiguousarray(np.tile(bq, 4).astype(np.float32))  # [128]
    bk4 = np.ascontiguousarray(np.tile(bk, 4).astype(np.float32))
    gbv = (gamma * bv).astype(np.float32)[:, None]  # [C,1]

    x_f16 = x.astype(np.float16)  # [B,C,N]
    in_maps = []
    for core in range(NCORES):
        b, h = divmod(core, 2)
        xq = x[b][:, h * NQ:(h + 1) * NQ] + gbv
        in_maps.append(
            {
                "xh": x_f16[b],
                "xqh": np.ascontiguousarray(x_f16[b][:, h * NQ:(h + 1) * NQ]),
                "xq": np.ascontiguousarray(xq, dtype=np.float32),
                "wqt4": wqt4,
                "wkt4": wkt4,
                "wvt": wvt,
                "bq4": bq4,
                "bk4": bk4,
            }
        )
    return in_maps


def assemble(results):
    out = np.empty((B, C, N), np.float32)
    for core in range(NCORES):
        b, h = divmod(core, 2)
        out[b][:, h * NQ:(h + 1) * NQ] = results[core]["out"]
    return out.reshape(B, C, HH, WW)


def kernel(**inputs):
    from concourse.bass_utils import run_bass_kernel_spmd

    nc = _get_prog()
    in_maps = make_in_maps(inputs)
    res = run_bass_kernel_spmd(nc, in_maps, core_ids=list(range(NCORES)))
    return assemble(res.results)
